# revision 19
# baseline (speedup 1.0000x reference)
"""Trainium2 Bass kernel for nn_Block_36661840839576 (dense transformer block).

Block: x -> LN1(over time) -> 16-head causal attention -> +residual
         -> LN2(over time) -> FFN(1024->1024 relu 1024->1024) -> +residual
Shapes: B=2, T=2048, C=1024, H=16, head_size=64. All fp32 I/O.

Sharding (8 cores): cores 0-3 handle batch 0, cores 4-7 batch 1.
Within a batch group of 4 cores: tensor-parallel over heads for attention
(4 heads/core over full T), then a ReduceScatter combines the per-head-group
proj partials and hands each core its own T-quarter (512 rows) for the FFN.
LN2 statistics (sums over the time axis) are combined with a tiny AllReduce.
Everything on-chip runs in a channels-on-partitions ("CT") layout so both
time-axis layernorms reduce along the free axis and all matmuls contract on
the partition axis; the host pre-transposes x and post-transposes y.

Matmuls run in bf16 (fp32 accumulation in PSUM); softmax runs exp in fp32 in
-> bf16 out without max-subtraction (logits are O(1) here: scale = C**-0.5).
"""

import sys

sys.path.insert(0, "/opt/trn_rl_repo")

import numpy as np
import ml_dtypes

import concourse.bass as bass
import concourse.tile as tile
from concourse import bacc
import concourse.mybir as mybir
from concourse.bass_utils import run_bass_kernel_spmd

F32 = mybir.dt.float32
BF16 = mybir.dt.bfloat16
AF = mybir.ActivationFunctionType
ALU = mybir.AluOpType

B, T, C, H = 2, 2048, 1024, 16
HS = C // H            # 64 head size
G = 4                  # cores per batch group
HPC = H // G           # 4 heads per core
NPAIR = HPC // 2       # 2 head pairs per core
TQ = T // G            # 512: t-quarter handled per core post-RS
P = 128
CT = C // P            # 8 channel tiles
TT = T // P            # 16 time tiles
NB = T // TQ           # 4 t1 blocks of 512
EPS = 1e-5
SCALE = float(C) ** -0.5
NDOF = float(T) / float(T - 1)    # population->unbiased var correction

# vec pack indices (host packs per-channel vectors as [128, NVEC, 8])
VG1, VBE1, VBPJ, VG2, VBE2, VB1, VB2 = range(7)
NVEC = 7


def device_body(tc, io, replica_groups, upto="full"):
    """Emit the per-core program. io: dict name -> dram AP.
    upto: debug bisect — stop after "qkv" / "attn" / "rs" / "ln2", writing
    intermediates to y instead of the final result."""
    nc = tc.nc
    from contextlib import ExitStack

    def dbg_write(srcs):
        # write up to 8 [128, 512]-shaped f32-castable APs to y
        with tc.tile_pool(name="dbg", bufs=2) as dbg:
            for i in range(CT):
                t = dbg.tile([P, TQ], F32, tag="dbg")
                if i < len(srcs) and srcs[i] is not None:
                    nc.vector.tensor_copy(t, srcs[i])
                else:
                    nc.vector.memset(t, 0.0)
                nc.sync.dma_start(io["y"][i * P : (i + 1) * P, :], t)

    with ExitStack() as top:
        const = top.enter_context(tc.tile_pool(name="const", bufs=1))
        dram = top.enter_context(tc.tile_pool(name="dram", bufs=1, space="DRAM"))

        eps_t = const.tile([P, 1], F32)
        nc.vector.memset(eps_t, EPS)
        ones64 = const.tile([1, 64], F32)
        nc.vector.memset(ones64, 1.0)
        vecs = const.tile([P, NVEC, CT], F32)
        nc.sync.dma_start(vecs, io["vecs"])

        def vcol(vi, ct):
            return vecs[:, vi, ct : ct + 1]

        # long-lived activation buffers
        qkvo = top.enter_context(tc.tile_pool(name="qkvo", bufs=1))
        q_sb = qkvo.tile([P, NPAIR, T], BF16)
        k_sb = qkvo.tile([P, NPAIR, T], BF16)
        v_sb = qkvo.tile([P, TT, HPC, 65], BF16)   # [t2 in tile, t2 tile, head, hs+1]
        o_sb = qkvo.tile([P, NPAIR, T], BF16)      # normalized attn out, head-pair stacked
        nc.vector.memset(v_sb[:, :, :, 64:65], 1.0)

        wp_sb = const.tile([P, 2, C], BF16)        # proj weights [d-in-tile, d-tile, cc]
        nc.sync.dma_start(wp_sb, io["wp"].rearrange("a p c -> p a c"))

        xq_pool = top.enter_context(tc.tile_pool(name="xq", bufs=1))
        xq_sb = [xq_pool.tile([P, TQ], F32, name=f"xq{i}") for i in range(CT)]
        for ct in range(CT):
            nc.sync.dma_start(xq_sb[ct], io["xq"][ct * P : (ct + 1) * P, :])

        # ---------------- Phase A+B: load x, LN1 (in place), QKV ----------------
        with ExitStack() as ph:
            xh_pool = ph.enter_context(tc.tile_pool(name="xh", bufs=1))
            wqkv = ph.enter_context(tc.tile_pool(name="wqkv", bufs=1))
            stat = ph.enter_context(tc.tile_pool(name="stat", bufs=2))
            mmps = ph.enter_context(tc.tile_pool(name="mmps", bufs=3, space="PSUM"))

            xh = [xh_pool.tile([P, T], BF16, name=f"xh{i}") for i in range(CT)]
            wq_sb = [wqkv.tile([P, NPAIR, P], BF16, name=f"wq{i}") for i in range(CT)]
            wk_sb = [wqkv.tile([P, NPAIR, P], BF16, name=f"wk{i}") for i in range(CT)]
            wv_sb = [wqkv.tile([P, HPC * HS], BF16, name=f"wv{i}") for i in range(CT)]
            for ct in range(CT):
                sl = slice(ct * P, (ct + 1) * P)
                nc.sync.dma_start(xh[ct], io["xt"][sl, :])
                nc.sync.dma_start(wq_sb[ct], io["wq"][sl])
                nc.sync.dma_start(wk_sb[ct], io["wk"][sl])
                nc.sync.dma_start(wv_sb[ct], io["wv"][sl])

            # LN1 per channel tile: stats over free (time) axis, then in-place affine
            for ct in range(CT):
                st = stat.tile([P, T // 512, 6], F32, tag="st")
                for ch in range(T // 512):
                    nc.vector.bn_stats(st[:, ch, :], xh[ct][:, ch * 512 : (ch + 1) * 512])
                mv = stat.tile([P, 2], F32, tag="mv")
                nc.vector.bn_aggr(mv, st)
                sd = stat.tile([P, 1], F32, tag="sd")
                nc.scalar.activation(sd, mv[:, 1:2], AF.Sqrt, bias=eps_t, scale=NDOF)
                rstd = stat.tile([P, 1], F32, tag="rstd")
                nc.vector.reciprocal(rstd, sd)
                a_t = stat.tile([P, 1], F32, tag="a_t")
                nc.vector.tensor_mul(a_t, rstd, vcol(VG1, ct))
                b_t = stat.tile([P, 1], F32, tag="b_t")
                nc.vector.tensor_mul(b_t, mv[:, 0:1], a_t)
                nc.vector.tensor_sub(b_t, vcol(VBE1, ct), b_t)
                nc.scalar.activation(xh[ct], xh[ct], AF.Identity, bias=b_t, scale=a_t)

            # Q, K: out[d(2 heads stacked), t] per pair; V: out[t, 4*64] per t2 tile
            for pair in range(NPAIR):
                for tb in range(NB):
                    tsl = slice(tb * TQ, (tb + 1) * TQ)
                    psq = mmps.tile([P, TQ], F32, tag="ps")
                    for ct in range(CT):
                        nc.tensor.matmul(psq, wq_sb[ct][:, pair, :], xh[ct][:, tsl],
                                         start=(ct == 0), stop=(ct == CT - 1))
                    nc.vector.tensor_copy(q_sb[:, pair, tsl], psq)
                    psk = mmps.tile([P, TQ], F32, tag="ps")
                    for ct in range(CT):
                        nc.tensor.matmul(psk, wk_sb[ct][:, pair, :], xh[ct][:, tsl],
                                         start=(ct == 0), stop=(ct == CT - 1))
                    nc.vector.tensor_copy(k_sb[:, pair, tsl], psk)
            for tt in range(TT):
                tsl = slice(tt * P, (tt + 1) * P)
                psv = mmps.tile([P, HPC * HS], F32, tag="psv")
                for ct in range(CT):
                    nc.tensor.matmul(psv, xh[ct][:, tsl], wv_sb[ct],
                                     start=(ct == 0), stop=(ct == CT - 1))
                nc.vector.tensor_copy(v_sb[:, tt, :, 0:64],
                                      psv.rearrange("p (h d) -> p h d", h=HPC))

            if upto == "qkv":
                dbg_write([q_sb[:, 0, 0:TQ], q_sb[:, 1, 0:TQ],
                           k_sb[:, 0, 0:TQ], k_sb[:, 1, 0:TQ],
                           v_sb[:, 0:2, :, 0:64], v_sb[:, 2:4, :, 0:64],
                           xh[0][:, 0:TQ], xh[7][:, 0:TQ]])
                return

        # ---------------- Phase C: causal attention per head ----------------
        with ExitStack() as ph:
            ppool = ph.enter_context(tc.tile_pool(name="pp", bufs=3))
            sps = ph.enter_context(tc.tile_pool(name="sps", bufs=3, space="PSUM"))
            ops_pool = ph.enter_context(tc.tile_pool(name="opsp", bufs=2, space="PSUM"))
            rbp = ph.enter_context(tc.tile_pool(name="rbp", bufs=2, space="PSUM"))
            rcp = ph.enter_context(tc.tile_pool(name="rcp", bufs=2))

            for h in range(HPC):
                pair, half = h // 2, h % 2
                psl = slice(64 * half, 64 * half + 64)
                for j in range(NB):
                    jsl = slice(j * TQ, (j + 1) * TQ)
                    ops = ops_pool.tile([65, TQ], F32, tag="o")
                    nctx = 4 * (j + 1)
                    for t2 in range(nctx):
                        sp = sps.tile([P, TQ], F32, tag="s")
                        nc.tensor.matmul(sp, k_sb[psl, pair, t2 * P : (t2 + 1) * P],
                                         q_sb[psl, pair, jsl], start=True, stop=True)
                        pt = ppool.tile([P, TQ], BF16, tag="P")
                        nc.scalar.activation(pt, sp, AF.Exp, scale=SCALE)
                        if t2 >= 4 * j:   # diagonal-crossing tile: zero where t2 > t1
                            nc.gpsimd.affine_select(
                                pt, pt, pattern=[[1, TQ]], base=j * TQ - t2 * P,
                                channel_multiplier=-1, compare_op=ALU.is_ge,
                                fill=0.0)
                        nc.tensor.matmul(ops, v_sb[:, t2, h, :], pt,
                                         start=(t2 == 0), stop=(t2 == nctx - 1))
                    rc = rcp.tile([1, TQ], F32, tag="rc")
                    nc.vector.reciprocal(rc, ops[64:65, :])
                    rb = rbp.tile([64, TQ], F32, tag="rb")
                    nc.tensor.matmul(rb, ones64, rc, start=True, stop=True)
                    rb_sb = rcp.tile([64, TQ], F32, tag="rb_sb")
                    nc.scalar.copy(rb_sb, rb[0:64, :])
                    nc.vector.tensor_mul(o_sb[psl, pair, jsl], ops[0:64, :], rb_sb)

        if upto == "attn":
            dbg_write([o_sb[:, 0, 0:TQ], o_sb[:, 1, 0:TQ],
                       o_sb[:, 0, TQ:2*TQ], o_sb[:, 1, TQ:2*TQ],
                       o_sb[:, 0, 3*TQ:4*TQ], o_sb[:, 1, 3*TQ:4*TQ]])
            return

        # ---------------- Phase D: proj partials + ReduceScatter ----------------
        rs_in = dram.tile([G, C, TQ], F32)
        rs_out = dram.tile([C, TQ], F32)
        with ExitStack() as ph:
            pjps = ph.enter_context(tc.tile_pool(name="pjps", bufs=3, space="PSUM"))
            part_pool = ph.enter_context(tc.tile_pool(name="part", bufs=3))
            for j in range(NB):
                jsl = slice(j * TQ, (j + 1) * TQ)
                for cc in range(CT):
                    pp = pjps.tile([P, TQ], F32, tag="pp")
                    for dt in range(2):
                        nc.tensor.matmul(pp, wp_sb[:, dt, cc * P : (cc + 1) * P],
                                         o_sb[:, dt, jsl], start=(dt == 0), stop=(dt == 1))
                    part = part_pool.tile([P, TQ], F32, tag="part")
                    nc.scalar.copy(part, pp)
                    nc.sync.dma_start(rs_in[j, cc * P : (cc + 1) * P, :], part)
        nc.gpsimd.collective_compute(
            "ReduceScatter", ALU.add, replica_groups=replica_groups,
            ins=[rs_in.opt()], outs=[rs_out.opt()])

        if upto == "rs":
            with tc.tile_pool(name="dbg2", bufs=2) as dbg:
                for i in range(CT):
                    t = dbg.tile([P, TQ], F32, tag="dbg")
                    nc.sync.dma_start(t, rs_out[i * P : (i + 1) * P, :])
                    nc.sync.dma_start(io["y"][i * P : (i + 1) * P, :], t)
            return

        # ---------------- Phase E: residual + LN2 stats + AllReduce ----------------
        ar_in = dram.tile([P, 2 * CT], F32)
        ar_out = dram.tile([P, 2 * CT], F32)
        xa_pool = top.enter_context(tc.tile_pool(name="xa", bufs=1))
        xa_sb = [xa_pool.tile([P, TQ], F32, name=f"xa{i}") for i in range(CT)]
        h2_sb = [xa_pool.tile([P, TQ], BF16, name=f"h2{i}") for i in range(CT)]
        with ExitStack() as ph:
            rsq_pool = ph.enter_context(tc.tile_pool(name="rsq", bufs=3))
            sq_pool = ph.enter_context(tc.tile_pool(name="sq", bufs=2))
            stat2 = ph.enter_context(tc.tile_pool(name="stat2", bufs=2))
            stats = const.tile([P, 2 * CT], F32)
            for ct in range(CT):
                rsq = rsq_pool.tile([P, TQ], F32, tag="rsq")
                nc.sync.dma_start(rsq, rs_out[ct * P : (ct + 1) * P, :])
                tsum = rsq_pool.tile([P, TQ], F32, tag="tsum")
                nc.vector.tensor_add(tsum, rsq, xq_sb[ct])
                nc.scalar.activation(xa_sb[ct], tsum, AF.Identity,
                                     bias=vcol(VBPJ, ct), scale=1.0)
                nc.vector.tensor_reduce(out=stats[:, 2 * ct : 2 * ct + 1],
                                        in_=xa_sb[ct], op=ALU.add,
                                        axis=mybir.AxisListType.X)
                sq = sq_pool.tile([P, TQ], F32, tag="sq")
                nc.vector.tensor_mul(sq, xa_sb[ct], xa_sb[ct])
                nc.vector.tensor_reduce(out=stats[:, 2 * ct + 1 : 2 * ct + 2],
                                        in_=sq, op=ALU.add,
                                        axis=mybir.AxisListType.X)
            if upto == "xa":
                dbg_write([xa_sb[0], xa_sb[7]])
                return
            nc.sync.dma_start(ar_in, stats)
            nc.gpsimd.collective_compute(
                "AllReduce", ALU.add, replica_groups=replica_groups,
                ins=[ar_in.opt()], outs=[ar_out.opt()])
            statsr = const.tile([P, 2 * CT], F32)
            nc.sync.dma_start(statsr, ar_out)

            for ct in range(CT):
                m_t = stat2.tile([P, 1], F32, tag="m_t")
                nc.scalar.mul(m_t, statsr[:, 2 * ct : 2 * ct + 1], 1.0 / T)
                msq = stat2.tile([P, 1], F32, tag="msq")
                nc.vector.tensor_mul(msq, m_t, m_t)
                var = stat2.tile([P, 1], F32, tag="var")
                # var*(T-1) = sx2 - T*m^2  via Identity(msq * -T + bias=sx2)
                nc.scalar.activation(var, msq, AF.Identity, scale=-float(T),
                                     bias=statsr[:, 2 * ct + 1 : 2 * ct + 2])
                sd2 = stat2.tile([P, 1], F32, tag="sd2")
                nc.scalar.activation(sd2, var, AF.Sqrt, bias=eps_t, scale=1.0 / (T - 1))
                rstd2 = stat2.tile([P, 1], F32, tag="rstd2")
                nc.vector.reciprocal(rstd2, sd2)
                a2 = stat2.tile([P, 1], F32, tag="a2")
                nc.vector.tensor_mul(a2, rstd2, vcol(VG2, ct))
                b2t = stat2.tile([P, 1], F32, tag="b2t")
                nc.vector.tensor_mul(b2t, m_t, a2)
                nc.vector.tensor_sub(b2t, vcol(VBE2, ct), b2t)
                nc.scalar.activation(h2_sb[ct], xa_sb[ct], AF.Identity, bias=b2t, scale=a2)

        if upto == "ln2":
            dbg_write([xa_sb[0], xa_sb[7], h2_sb[0], h2_sb[7]])
            return

        # ---------------- Phase F: FFN + residual + output ----------------
        with ExitStack() as ph:
            wffn = ph.enter_context(tc.tile_pool(name="wffn", bufs=1))
            a1_pool = ph.enter_context(tc.tile_pool(name="a1", bufs=1))
            ffps = ph.enter_context(tc.tile_pool(name="ffps", bufs=3, space="PSUM"))
            y_pool = ph.enter_context(tc.tile_pool(name="y", bufs=3))
            w1_sb = [wffn.tile([P, C], BF16, name=f"w1_{i}") for i in range(CT)]
            w2_sb = [wffn.tile([P, C], BF16, name=f"w2_{i}") for i in range(CT)]
            for ct in range(CT):
                sl = slice(ct * P, (ct + 1) * P)
                nc.sync.dma_start(w1_sb[ct], io["w1"][sl, :])
                nc.sync.dma_start(w2_sb[ct], io["w2"][sl, :])
            a1_sb = [a1_pool.tile([P, TQ], BF16, name=f"a1_{i}") for i in range(CT)]
            for jt in range(CT):
                ps1 = ffps.tile([P, TQ], F32, tag="f1")
                for ct in range(CT):
                    nc.tensor.matmul(ps1, w1_sb[ct][:, jt * P : (jt + 1) * P], h2_sb[ct],
                                     start=(ct == 0), stop=(ct == CT - 1))
                nc.scalar.activation(a1_sb[jt], ps1, AF.Relu, bias=vcol(VB1, jt), scale=1.0)
            for cc in range(CT):
                ps2 = ffps.tile([P, TQ], F32, tag="f2")
                for jt in range(CT):
                    nc.tensor.matmul(ps2, w2_sb[jt][:, cc * P : (cc + 1) * P], a1_sb[jt],
                                     start=(jt == 0), stop=(jt == CT - 1))
                yb = y_pool.tile([P, TQ], F32, tag="yb")
                nc.scalar.activation(yb, ps2, AF.Identity, bias=vcol(VB2, cc), scale=1.0)
                yt = y_pool.tile([P, TQ], F32, tag="y")
                nc.vector.tensor_add(yt, yb, xa_sb[cc])
                nc.sync.dma_start(io["y"][cc * P : (cc + 1) * P, :], yt)


def build_module(num_devices=8, upto="full"):
    nc = bacc.Bacc("TRN2", target_bir_lowering=False, num_devices=num_devices)
    io = {
        "xt": nc.dram_tensor("xt", [C, T], BF16, kind="ExternalInput").ap(),
        "xq": nc.dram_tensor("xq", [C, TQ], F32, kind="ExternalInput").ap(),
        "wq": nc.dram_tensor("wq", [C, NPAIR, P], BF16, kind="ExternalInput").ap(),
        "wk": nc.dram_tensor("wk", [C, NPAIR, P], BF16, kind="ExternalInput").ap(),
        "wv": nc.dram_tensor("wv", [C, HPC * HS], BF16, kind="ExternalInput").ap(),
        "wp": nc.dram_tensor("wp", [2, P, C], BF16, kind="ExternalInput").ap(),
        "w1": nc.dram_tensor("w1", [C, C], BF16, kind="ExternalInput").ap(),
        "w2": nc.dram_tensor("w2", [C, C], BF16, kind="ExternalInput").ap(),
        "vecs": nc.dram_tensor("vecs", [P, NVEC, CT], F32, kind="ExternalInput").ap(),
        "y": nc.dram_tensor("y", [C, TQ], F32, kind="ExternalOutput").ap(),
    }
    groups = [list(range(g * G, (g + 1) * G)) for g in range(num_devices // G)]
    with tile.TileContext(nc) as tc:
        device_body(tc, io, groups, upto=upto)
    nc.compile()
    return nc


def prep_core_inputs(inputs, c):
    """Host-side shard prep for core c. b = c//4 (batch), g = c%4 (head group & t-quarter)."""
    b, g = c // G, c % G
    bf = ml_dtypes.bfloat16
    x = np.asarray(inputs["x"][b], dtype=np.float32)          # [T, C]
    xt = np.ascontiguousarray(x.T)                            # [C, T]
    wq4 = np.asarray(inputs["Wq"][g * HPC:(g + 1) * HPC], np.float32)   # [4, C, 64]
    wk4 = np.asarray(inputs["Wk"][g * HPC:(g + 1) * HPC], np.float32)
    wv4 = np.asarray(inputs["Wv"][g * HPC:(g + 1) * HPC], np.float32)
    vec_list = [inputs["g1"], inputs["beta1"], inputs["bproj"],
                inputs["g2"], inputs["beta2"], inputs["b1"], inputs["b2"]]
    vecs = np.stack([np.asarray(v, np.float32).reshape(CT, P).T for v in vec_list],
                    axis=1)                                   # [128, NVEC, 8]
    return {
        "xt": xt.astype(bf),
        "xq": np.ascontiguousarray(xt[:, g * TQ:(g + 1) * TQ]),
        "wq": np.ascontiguousarray(wq4.transpose(1, 0, 2).reshape(C, NPAIR, P)).astype(bf),
        "wk": np.ascontiguousarray(wk4.transpose(1, 0, 2).reshape(C, NPAIR, P)).astype(bf),
        "wv": np.ascontiguousarray(wv4.transpose(1, 0, 2).reshape(C, HPC * HS)).astype(bf),
        "wp": np.ascontiguousarray(
            np.asarray(inputs["Wproj"], np.float32)[g * HPC * HS:(g + 1) * HPC * HS, :]
            .reshape(2, P, C)).astype(bf),
        "w1": np.asarray(inputs["W1"], np.float32).astype(bf),
        "w2": np.asarray(inputs["W2"], np.float32).astype(bf),
        "vecs": np.ascontiguousarray(vecs),
    }


_MODULE = None
TRACE = False          # set True (e.g. from test.py) to capture an NTFF profile
LAST_RESULTS = None    # BassKernelResults of the most recent run


def kernel(**inputs) -> np.ndarray:
    global _MODULE, LAST_RESULTS
    if _MODULE is None:
        _MODULE = build_module(num_devices=8)
    nc = _MODULE
    in_maps = [prep_core_inputs(inputs, c) for c in range(8)]
    res = run_bass_kernel_spmd(nc, in_maps, core_ids=list(range(8)), trace=TRACE)
    LAST_RESULTS = res
    out = np.zeros((B, T, C), np.float32)
    for c in range(8):
        b, g = c // G, c % G
        out[b, g * TQ:(g + 1) * TQ, :] = res.results[c]["y"].T
    return out


if __name__ == "__main__":
    import reference
    inputs = {k: np.asarray(v) for k, v in reference.setup_inputs().items()}
    expected = np.asarray(reference.reference(**inputs))
    actual = kernel(**inputs)
    rel = np.linalg.norm(actual - expected) / np.linalg.norm(expected)
    print("Relative error:", rel)


# revision 22
# speedup vs baseline: 1.2411x; 1.2411x over previous
"""Trainium2 Bass kernel for nn_Block_36661840839576 (dense transformer block).

Block: x -> LN1(over time) -> 16-head causal attention -> +residual
         -> LN2(over time) -> FFN(1024->1024 relu 1024->1024) -> +residual
Shapes: B=2, T=2048, C=1024, H=16, head_size=64. All fp32 I/O.

Sharding (8 cores): cores 0-3 handle batch 0, cores 4-7 batch 1.
Within a batch group of 4 cores: tensor-parallel over heads for attention
(4 heads/core over full T), then a ReduceScatter combines the per-head-group
proj partials and hands each core its own T-quarter (512 rows) for the FFN.
LN2 statistics (sums over the time axis) are combined with a tiny AllReduce.
Everything on-chip runs in a channels-on-partitions ("CT") layout so both
time-axis layernorms reduce along the free axis and all matmuls contract on
the partition axis; the host pre-transposes x and post-transposes y.

Matmuls run in bf16 (fp32 accumulation in PSUM); softmax runs exp in fp32 in
-> bf16 out without max-subtraction (logits are O(1) here: scale = C**-0.5).
"""

import sys

sys.path.insert(0, "/opt/trn_rl_repo")

import numpy as np
import ml_dtypes

import concourse.bass as bass
import concourse.tile as tile
from concourse import bacc
import concourse.mybir as mybir
from concourse.bass_utils import run_bass_kernel_spmd

F32 = mybir.dt.float32
BF16 = mybir.dt.bfloat16
AF = mybir.ActivationFunctionType
ALU = mybir.AluOpType

B, T, C, H = 2, 2048, 1024, 16
HS = C // H            # 64 head size
G = 4                  # cores per batch group
HPC = H // G           # 4 heads per core
NPAIR = HPC // 2       # 2 head pairs per core
TQ = T // G            # 512: t-quarter handled per core post-RS
P = 128
CT = C // P            # 8 channel tiles
TT = T // P            # 16 time tiles
NB = T // TQ           # 4 t1 blocks of 512
EPS = 1e-5
SCALE = float(C) ** -0.5
NDOF = float(T) / float(T - 1)    # population->unbiased var correction

# vec pack indices (host packs per-channel vectors as [128, NVEC, 8])
VG1, VBE1, VBPJ, VG2, VBE2, VB1, VB2 = range(7)
NVEC = 7


def device_body(tc, io, replica_groups, upto="full"):
    """Emit the per-core program. io: dict name -> dram AP.
    upto: debug bisect — stop after "qkv" / "attn" / "rs" / "ln2", writing
    intermediates to y instead of the final result."""
    nc = tc.nc
    from contextlib import ExitStack

    def dbg_write(srcs):
        # write up to 8 [128, 512]-shaped f32-castable APs to y
        with tc.tile_pool(name="dbg", bufs=2) as dbg:
            for i in range(CT):
                t = dbg.tile([P, TQ], F32, tag="dbg")
                if i < len(srcs) and srcs[i] is not None:
                    nc.vector.tensor_copy(t, srcs[i])
                else:
                    nc.vector.memset(t, 0.0)
                nc.sync.dma_start(io["y"][i * P : (i + 1) * P, :], t)

    with ExitStack() as top:
        const = top.enter_context(tc.tile_pool(name="const", bufs=1))
        dram = top.enter_context(tc.tile_pool(name="dram", bufs=1, space="DRAM"))

        eps_t = const.tile([P, 1], F32)
        nc.vector.memset(eps_t, EPS)
        ones64 = const.tile([1, 64], F32)
        nc.vector.memset(ones64, 1.0)
        vecs = const.tile([P, NVEC, CT], F32)
        nc.sync.dma_start(vecs, io["vecs"])

        def vcol(vi, ct):
            return vecs[:, vi, ct : ct + 1]

        # long-lived activation buffers
        qkvo = top.enter_context(tc.tile_pool(name="qkvo", bufs=1))
        q_sb = qkvo.tile([P, NPAIR, T], BF16)
        k_sb = qkvo.tile([P, NPAIR, T], BF16)
        v_sb = qkvo.tile([P, TT, HPC, 65], BF16)   # [t2 in tile, t2 tile, head, hs+1]
        o_sb = qkvo.tile([P, NPAIR, T], BF16)      # normalized attn out, head-pair stacked
        nc.vector.memset(v_sb[:, :, :, 64:65], 1.0)

        wp_sb = const.tile([P, 2, C], BF16)        # proj weights [d-in-tile, d-tile, cc]
        nc.sync.dma_start(wp_sb, io["wp"].rearrange("a p c -> p a c"))

        xq_pool = top.enter_context(tc.tile_pool(name="xq", bufs=1))
        xq_sb = [xq_pool.tile([P, TQ], F32, name=f"xq{i}") for i in range(CT)]
        for ct in range(CT):
            nc.sync.dma_start(xq_sb[ct], io["xq"][ct * P : (ct + 1) * P, :])

        # ---------------- Phase A+B: load x, LN1 (in place), QKV ----------------
        with ExitStack() as ph:
            xh_pool = ph.enter_context(tc.tile_pool(name="xh", bufs=1))
            wqkv = ph.enter_context(tc.tile_pool(name="wqkv", bufs=1))
            stat = ph.enter_context(tc.tile_pool(name="stat", bufs=2))
            mmps = ph.enter_context(tc.tile_pool(name="mmps", bufs=3, space="PSUM"))

            xh = [xh_pool.tile([P, T], BF16, name=f"xh{i}") for i in range(CT)]
            wq_sb = [wqkv.tile([P, NPAIR, P], BF16, name=f"wq{i}") for i in range(CT)]
            wk_sb = [wqkv.tile([P, NPAIR, P], BF16, name=f"wk{i}") for i in range(CT)]
            wv_sb = [wqkv.tile([P, HPC * HS], BF16, name=f"wv{i}") for i in range(CT)]
            for ct in range(CT):
                sl = slice(ct * P, (ct + 1) * P)
                nc.sync.dma_start(xh[ct], io["xt"][sl, :])
                nc.sync.dma_start(wq_sb[ct], io["wq"][sl])
                nc.sync.dma_start(wk_sb[ct], io["wk"][sl])
                nc.sync.dma_start(wv_sb[ct], io["wv"][sl])

            # LN1 per channel tile: stats over free (time) axis, then in-place affine
            for ct in range(CT):
                st = stat.tile([P, T // 512, 6], F32, tag="st")
                for ch in range(T // 512):
                    nc.vector.bn_stats(st[:, ch, :], xh[ct][:, ch * 512 : (ch + 1) * 512])
                mv = stat.tile([P, 2], F32, tag="mv")
                nc.vector.bn_aggr(mv, st)
                sd = stat.tile([P, 1], F32, tag="sd")
                nc.scalar.activation(sd, mv[:, 1:2], AF.Sqrt, bias=eps_t, scale=NDOF)
                rstd = stat.tile([P, 1], F32, tag="rstd")
                nc.vector.reciprocal(rstd, sd)
                a_t = stat.tile([P, 1], F32, tag="a_t")
                nc.vector.tensor_mul(a_t, rstd, vcol(VG1, ct))
                b_t = stat.tile([P, 1], F32, tag="b_t")
                nc.vector.tensor_mul(b_t, mv[:, 0:1], a_t)
                nc.vector.tensor_sub(b_t, vcol(VBE1, ct), b_t)
                nc.scalar.activation(xh[ct], xh[ct], AF.Identity, bias=b_t, scale=a_t)

            # Q, K: out[d(2 heads stacked), t] per pair; V: out[t, 4*64] per t2 tile
            for pair in range(NPAIR):
                for tb in range(NB):
                    tsl = slice(tb * TQ, (tb + 1) * TQ)
                    psq = mmps.tile([P, TQ], F32, tag="ps")
                    for ct in range(CT):
                        nc.tensor.matmul(psq, wq_sb[ct][:, pair, :], xh[ct][:, tsl],
                                         start=(ct == 0), stop=(ct == CT - 1))
                    nc.vector.tensor_copy(q_sb[:, pair, tsl], psq)
                    psk = mmps.tile([P, TQ], F32, tag="ps")
                    for ct in range(CT):
                        nc.tensor.matmul(psk, wk_sb[ct][:, pair, :], xh[ct][:, tsl],
                                         start=(ct == 0), stop=(ct == CT - 1))
                    nc.vector.tensor_copy(k_sb[:, pair, tsl], psk)
            for tt in range(TT):
                tsl = slice(tt * P, (tt + 1) * P)
                psv = mmps.tile([P, HPC * HS], F32, tag="psv")
                for ct in range(CT):
                    nc.tensor.matmul(psv, xh[ct][:, tsl], wv_sb[ct],
                                     start=(ct == 0), stop=(ct == CT - 1))
                nc.vector.tensor_copy(v_sb[:, tt, :, 0:64],
                                      psv.rearrange("p (h d) -> p h d", h=HPC))

            if upto == "qkv":
                dbg_write([q_sb[:, 0, 0:TQ], q_sb[:, 1, 0:TQ],
                           k_sb[:, 0, 0:TQ], k_sb[:, 1, 0:TQ],
                           v_sb[:, 0:2, :, 0:64], v_sb[:, 2:4, :, 0:64],
                           xh[0][:, 0:TQ], xh[7][:, 0:TQ]])
                return

        # ---------------- Phase C: causal attention per head ----------------
        with ExitStack() as ph:
            ppool = ph.enter_context(tc.tile_pool(name="pp", bufs=3))
            sps = ph.enter_context(tc.tile_pool(name="sps", bufs=3, space="PSUM"))
            ops_pool = ph.enter_context(tc.tile_pool(name="opsp", bufs=2, space="PSUM"))
            rbp = ph.enter_context(tc.tile_pool(name="rbp", bufs=2, space="PSUM"))
            rcp = ph.enter_context(tc.tile_pool(name="rcp", bufs=2))

            for h in range(HPC):
                pair, half = h // 2, h % 2
                psl = slice(64 * half, 64 * half + 64)
                for j in range(NB):
                    jsl = slice(j * TQ, (j + 1) * TQ)
                    ops = ops_pool.tile([65, TQ], F32, tag="o")
                    nctx = 4 * (j + 1)
                    for t2 in range(nctx):
                        sp = sps.tile([P, TQ], F32, tag="s")
                        nc.tensor.matmul(sp, k_sb[psl, pair, t2 * P : (t2 + 1) * P],
                                         q_sb[psl, pair, jsl], start=True, stop=True)
                        pt = ppool.tile([P, TQ], BF16, tag="P")
                        nc.scalar.activation(pt, sp, AF.Exp, scale=SCALE)
                        if t2 >= 4 * j:   # diagonal-crossing tile: zero where t2 > t1
                            nc.gpsimd.affine_select(
                                pt, pt, pattern=[[1, TQ]], base=j * TQ - t2 * P,
                                channel_multiplier=-1, compare_op=ALU.is_ge,
                                fill=0.0)
                        nc.tensor.matmul(ops, v_sb[:, t2, h, :], pt,
                                         start=(t2 == 0), stop=(t2 == nctx - 1))
                    rc = rcp.tile([1, TQ], F32, tag="rc")
                    nc.vector.reciprocal(rc, ops[64:65, :])
                    rb = rbp.tile([64, TQ], F32, tag="rb")
                    nc.tensor.matmul(rb, ones64, rc, start=True, stop=True)
                    rb_sb = rcp.tile([64, TQ], F32, tag="rb_sb")
                    nc.scalar.copy(rb_sb, rb[0:64, :])
                    nc.vector.tensor_mul(o_sb[psl, pair, jsl], ops[0:64, :], rb_sb)

        if upto == "attn":
            dbg_write([o_sb[:, 0, 0:TQ], o_sb[:, 1, 0:TQ],
                       o_sb[:, 0, TQ:2*TQ], o_sb[:, 1, TQ:2*TQ],
                       o_sb[:, 0, 3*TQ:4*TQ], o_sb[:, 1, 3*TQ:4*TQ]])
            return

        # ---------------- Phase D: proj partials + ReduceScatter ----------------
        # bf16 partials, split into two channel-halves so the first RS overlaps
        # with the second half's proj matmuls.
        HC = CT // 2
        rs_in = [dram.tile([G, HC * P, TQ], BF16, name=f"rsin{i}") for i in range(2)]
        rs_out = [dram.tile([HC * P, TQ], BF16, name=f"rsout{i}") for i in range(2)]
        with ExitStack() as ph:
            pjps = ph.enter_context(tc.tile_pool(name="pjps", bufs=3, space="PSUM"))
            part_pool = ph.enter_context(tc.tile_pool(name="part", bufs=3))
            for half in range(2):
                for cc in range(half * HC, (half + 1) * HC):
                    for j in range(NB):
                        jsl = slice(j * TQ, (j + 1) * TQ)
                        pp = pjps.tile([P, TQ], F32, tag="pp")
                        for dt in range(2):
                            nc.tensor.matmul(pp, wp_sb[:, dt, cc * P : (cc + 1) * P],
                                             o_sb[:, dt, jsl], start=(dt == 0), stop=(dt == 1))
                        part = part_pool.tile([P, TQ], BF16, tag="part")
                        nc.scalar.copy(part, pp)
                        nc.sync.dma_start(
                            rs_in[half][j, (cc - half * HC) * P : (cc - half * HC + 1) * P, :],
                            part)
                nc.gpsimd.collective_compute(
                    "ReduceScatter", ALU.add, replica_groups=replica_groups,
                    ins=[rs_in[half].opt()], outs=[rs_out[half].opt()])

        # ---------------- Phase E: residual + LN2 stats + AllReduce ----------------
        ar_in = dram.tile([P, 2 * CT], F32)
        ar_out = dram.tile([P, 2 * CT], F32)
        xa_pool = top.enter_context(tc.tile_pool(name="xa", bufs=1))
        xa_sb = [xa_pool.tile([P, TQ], F32, name=f"xa{i}") for i in range(CT)]
        h2_sb = [xa_pool.tile([P, TQ], BF16, name=f"h2{i}") for i in range(CT)]
        with ExitStack() as ph:
            rsq_pool = ph.enter_context(tc.tile_pool(name="rsq", bufs=3))
            sq_pool = ph.enter_context(tc.tile_pool(name="sq", bufs=2))
            stat2 = ph.enter_context(tc.tile_pool(name="stat2", bufs=2))
            stats = const.tile([P, 2 * CT], F32)
            for ct in range(CT):
                rsq = rsq_pool.tile([P, TQ], BF16, tag="rsq")
                nc.sync.dma_start(
                    rsq, rs_out[ct // HC][(ct % HC) * P : (ct % HC + 1) * P, :])
                tsum = rsq_pool.tile([P, TQ], F32, tag="tsum")
                nc.vector.tensor_add(tsum, rsq, xq_sb[ct])
                nc.scalar.activation(xa_sb[ct], tsum, AF.Identity,
                                     bias=vcol(VBPJ, ct), scale=1.0)
                nc.vector.tensor_reduce(out=stats[:, 2 * ct : 2 * ct + 1],
                                        in_=xa_sb[ct], op=ALU.add,
                                        axis=mybir.AxisListType.X)
                sq = sq_pool.tile([P, TQ], F32, tag="sq")
                nc.vector.tensor_mul(sq, xa_sb[ct], xa_sb[ct])
                nc.vector.tensor_reduce(out=stats[:, 2 * ct + 1 : 2 * ct + 2],
                                        in_=sq, op=ALU.add,
                                        axis=mybir.AxisListType.X)
            if upto == "xa":
                dbg_write([xa_sb[0], xa_sb[7]])
                return
            nc.sync.dma_start(ar_in, stats)
            nc.gpsimd.collective_compute(
                "AllReduce", ALU.add, replica_groups=replica_groups,
                ins=[ar_in.opt()], outs=[ar_out.opt()])
            statsr = const.tile([P, 2 * CT], F32)
            nc.sync.dma_start(statsr, ar_out)

            for ct in range(CT):
                m_t = stat2.tile([P, 1], F32, tag="m_t")
                nc.scalar.mul(m_t, statsr[:, 2 * ct : 2 * ct + 1], 1.0 / T)
                msq = stat2.tile([P, 1], F32, tag="msq")
                nc.vector.tensor_mul(msq, m_t, m_t)
                var = stat2.tile([P, 1], F32, tag="var")
                # var*(T-1) = sx2 - T*m^2  via Identity(msq * -T + bias=sx2)
                nc.scalar.activation(var, msq, AF.Identity, scale=-float(T),
                                     bias=statsr[:, 2 * ct + 1 : 2 * ct + 2])
                sd2 = stat2.tile([P, 1], F32, tag="sd2")
                nc.scalar.activation(sd2, var, AF.Sqrt, bias=eps_t, scale=1.0 / (T - 1))
                rstd2 = stat2.tile([P, 1], F32, tag="rstd2")
                nc.vector.reciprocal(rstd2, sd2)
                a2 = stat2.tile([P, 1], F32, tag="a2")
                nc.vector.tensor_mul(a2, rstd2, vcol(VG2, ct))
                b2t = stat2.tile([P, 1], F32, tag="b2t")
                nc.vector.tensor_mul(b2t, m_t, a2)
                nc.vector.tensor_sub(b2t, vcol(VBE2, ct), b2t)
                nc.scalar.activation(h2_sb[ct], xa_sb[ct], AF.Identity, bias=b2t, scale=a2)

        if upto == "ln2":
            dbg_write([xa_sb[0], xa_sb[7], h2_sb[0], h2_sb[7]])
            return

        # ---------------- Phase F: FFN + residual + output ----------------
        with ExitStack() as ph:
            wffn = ph.enter_context(tc.tile_pool(name="wffn", bufs=1))
            a1_pool = ph.enter_context(tc.tile_pool(name="a1", bufs=1))
            ffps = ph.enter_context(tc.tile_pool(name="ffps", bufs=3, space="PSUM"))
            y_pool = ph.enter_context(tc.tile_pool(name="y", bufs=3))
            w1_sb = [wffn.tile([P, C], BF16, name=f"w1_{i}") for i in range(CT)]
            w2_sb = [wffn.tile([P, C], BF16, name=f"w2_{i}") for i in range(CT)]
            for ct in range(CT):
                sl = slice(ct * P, (ct + 1) * P)
                nc.sync.dma_start(w1_sb[ct], io["w1"][sl, :])
                nc.sync.dma_start(w2_sb[ct], io["w2"][sl, :])
            a1_sb = [a1_pool.tile([P, TQ], BF16, name=f"a1_{i}") for i in range(CT)]
            for jt in range(CT):
                ps1 = ffps.tile([P, TQ], F32, tag="f1")
                for ct in range(CT):
                    nc.tensor.matmul(ps1, w1_sb[ct][:, jt * P : (jt + 1) * P], h2_sb[ct],
                                     start=(ct == 0), stop=(ct == CT - 1))
                nc.scalar.activation(a1_sb[jt], ps1, AF.Relu, bias=vcol(VB1, jt), scale=1.0)
            for cc in range(CT):
                ps2 = ffps.tile([P, TQ], F32, tag="f2")
                for jt in range(CT):
                    nc.tensor.matmul(ps2, w2_sb[jt][:, cc * P : (cc + 1) * P], a1_sb[jt],
                                     start=(jt == 0), stop=(jt == CT - 1))
                yb = y_pool.tile([P, TQ], F32, tag="yb")
                nc.scalar.activation(yb, ps2, AF.Identity, bias=vcol(VB2, cc), scale=1.0)
                yt = y_pool.tile([P, TQ], F32, tag="y")
                nc.vector.tensor_add(yt, yb, xa_sb[cc])
                nc.sync.dma_start(io["y"][cc * P : (cc + 1) * P, :], yt)


def build_module(num_devices=8, upto="full"):
    nc = bacc.Bacc("TRN2", target_bir_lowering=False, num_devices=num_devices)
    io = {
        "xt": nc.dram_tensor("xt", [C, T], BF16, kind="ExternalInput").ap(),
        "xq": nc.dram_tensor("xq", [C, TQ], F32, kind="ExternalInput").ap(),
        "wq": nc.dram_tensor("wq", [C, NPAIR, P], BF16, kind="ExternalInput").ap(),
        "wk": nc.dram_tensor("wk", [C, NPAIR, P], BF16, kind="ExternalInput").ap(),
        "wv": nc.dram_tensor("wv", [C, HPC * HS], BF16, kind="ExternalInput").ap(),
        "wp": nc.dram_tensor("wp", [2, P, C], BF16, kind="ExternalInput").ap(),
        "w1": nc.dram_tensor("w1", [C, C], BF16, kind="ExternalInput").ap(),
        "w2": nc.dram_tensor("w2", [C, C], BF16, kind="ExternalInput").ap(),
        "vecs": nc.dram_tensor("vecs", [P, NVEC, CT], F32, kind="ExternalInput").ap(),
        "y": nc.dram_tensor("y", [C, TQ], F32, kind="ExternalOutput").ap(),
    }
    groups = [list(range(g * G, (g + 1) * G)) for g in range(num_devices // G)]
    with tile.TileContext(nc) as tc:
        device_body(tc, io, groups, upto=upto)
    nc.compile()
    return nc


def prep_core_inputs(inputs, c):
    """Host-side shard prep for core c. b = c//4 (batch), g = c%4 (head group & t-quarter)."""
    b, g = c // G, c % G
    bf = ml_dtypes.bfloat16
    x = np.asarray(inputs["x"][b], dtype=np.float32)          # [T, C]
    xt = np.ascontiguousarray(x.T)                            # [C, T]
    wq4 = np.asarray(inputs["Wq"][g * HPC:(g + 1) * HPC], np.float32)   # [4, C, 64]
    wk4 = np.asarray(inputs["Wk"][g * HPC:(g + 1) * HPC], np.float32)
    wv4 = np.asarray(inputs["Wv"][g * HPC:(g + 1) * HPC], np.float32)
    vec_list = [inputs["g1"], inputs["beta1"], inputs["bproj"],
                inputs["g2"], inputs["beta2"], inputs["b1"], inputs["b2"]]
    vecs = np.stack([np.asarray(v, np.float32).reshape(CT, P).T for v in vec_list],
                    axis=1)                                   # [128, NVEC, 8]
    return {
        "xt": xt.astype(bf),
        "xq": np.ascontiguousarray(xt[:, g * TQ:(g + 1) * TQ]),
        "wq": np.ascontiguousarray(wq4.transpose(1, 0, 2).reshape(C, NPAIR, P)).astype(bf),
        "wk": np.ascontiguousarray(wk4.transpose(1, 0, 2).reshape(C, NPAIR, P)).astype(bf),
        "wv": np.ascontiguousarray(wv4.transpose(1, 0, 2).reshape(C, HPC * HS)).astype(bf),
        "wp": np.ascontiguousarray(
            np.asarray(inputs["Wproj"], np.float32)[g * HPC * HS:(g + 1) * HPC * HS, :]
            .reshape(2, P, C)).astype(bf),
        "w1": np.asarray(inputs["W1"], np.float32).astype(bf),
        "w2": np.asarray(inputs["W2"], np.float32).astype(bf),
        "vecs": np.ascontiguousarray(vecs),
    }


_MODULE = None
TRACE = False          # set True (e.g. from test.py) to capture an NTFF profile
LAST_RESULTS = None    # BassKernelResults of the most recent run


def kernel(**inputs) -> np.ndarray:
    global _MODULE, LAST_RESULTS
    if _MODULE is None:
        _MODULE = build_module(num_devices=8)
    nc = _MODULE
    in_maps = [prep_core_inputs(inputs, c) for c in range(8)]
    res = run_bass_kernel_spmd(nc, in_maps, core_ids=list(range(8)), trace=TRACE)
    LAST_RESULTS = res
    out = np.zeros((B, T, C), np.float32)
    for c in range(8):
        b, g = c // G, c % G
        out[b, g * TQ:(g + 1) * TQ, :] = res.results[c]["y"].T
    return out


if __name__ == "__main__":
    import reference
    inputs = {k: np.asarray(v) for k, v in reference.setup_inputs().items()}
    expected = np.asarray(reference.reference(**inputs))
    actual = kernel(**inputs)
    rel = np.linalg.norm(actual - expected) / np.linalg.norm(expected)
    print("Relative error:", rel)
